# revision 21
# baseline (speedup 1.0000x reference)
"""Single-head attention (b=4, s=4096, d=1024, h=128) on 8 Trainium2 NeuronCores.

Sharding: data-parallel over batch x query-halves -> 8 independent cores
(core c handles batch c//2, query rows [hq*2048, (hq+1)*2048) with hq = c%2).
K/V work is replicated per batch pair; no collectives.

Host prep per core: x[b].T (d-major) in bf16, sequence columns rotated so
the core's 2048 query rows come first (softmax over keys is permutation-
invariant); weights in bf16, pre-arranged to [128p, 8c*128h] rows so their
DMA runs 2KB-contiguous descriptors; 1/sqrt(h) folded into Wq. The kernel
returns outT [h, 2048] f32 per core; the host transposes back.

Device kernel (bf16 matmul operands, fp32 PSUM accumulation):
  projections (per 512-col block, interleaved with attention units):
    qT = wq.T @ xT [128h, 2048q]; kT = wk.T @ xT [128h, 4096k]
    v natural [k,h] computed directly (xt chunks stationary, wv moving) --
    no transpose pass; PSUM->SBUF copies on DVE (bf16 out), ACT exp-only
  attention unit (key-block kb, q-half h); both halves' oT live in PSUM:
    sc = kT[kb].T @ qT[half] [128k, 1024q] PSUM f32 (2x 512-wide matmuls)
    ex = exp(sc)  -- ONE 1024-wide ACT instr (table preloaded at t=0)
    oT[half] += v[kb].T' @ ex  (PSUM accumulate over kb)
    den: DVE bf16 chain-accumulate acc[h] += ex (last block folded into
    the den matmul instead)
  finale per half: den = ones.T @ (acc + ex_last) on PE (partition sum,
    broadcast via the all-ones stationary); DVE reciprocal+multiply and
    output DMA in 512-col chunks.

The emission plan rate-matches PE and ACT: exp (the co-bottleneck, ~73us
of ACT) starts ~14us in, V projections are deferred into pipeline gaps
(back-units gated on v-block availability), and the two q-halves finish
staggered so the first finale overlaps the second half's tail. Measured
steady-state: ~112us/core vs 179us for the fp32r two-phase baseline under
the same loop harness (rel err 1.01e-2 vs reference, gate 2e-2).
"""

import sys

sys.path.insert(0, "/opt/trn_rl_repo")

import numpy as np
import ml_dtypes

import concourse.mybir as mybir
from concourse import bacc
from concourse.bass_utils import run_bass_kernel_spmd

F32 = mybir.dt.float32
BF16 = mybir.dt.bfloat16

B = 4  # batch
D = 1024  # d_model
H = 128  # head size
S = 4096  # full sequence (keys)
SQ = 2048  # queries per core
DC = D // 128  # 8 d-chunks
NB = S // 512  # 8 column blocks for projections
KB = S // 128  # 32 key blocks
SC_BUFS = 2
EXP_BUFS = 8
XT_BUFS = 20


def build_attention_v2(loop_n=None, timing=False):
    """Build the v2 attention kernel; see module docstring.

    loop_n/timing wrap the body in an on-device For_i loop with Internal
    (device-resident, zero-filled) inputs for steady-state benchmarking.
    """
    from concourse.tile import TileContext

    nc = bacc.Bacc("TRN2", target_bir_lowering=False, debug=False)

    kind_in = "Internal" if timing else "ExternalInput"
    xT = nc.dram_tensor("xT", (D, S), BF16, kind=kind_in)
    # weights pre-arranged on host to [128p, DC*H] so the DMA runs 2KB rows
    wq = nc.dram_tensor("wq", (128, DC * H), BF16, kind=kind_in)
    wk = nc.dram_tensor("wk", (128, DC * H), BF16, kind=kind_in)
    wv = nc.dram_tensor("wv", (128, DC * H), BF16, kind=kind_in)
    outT = nc.dram_tensor("outT", (H, SQ), F32, kind="ExternalOutput")
    tick = (
        nc.dram_tensor("tick", (1, 16), F32, kind="ExternalInput") if timing else None
    )
    warm_d = nc.dram_tensor("warm_d", (1, 16), F32, kind="Internal")

    with TileContext(nc) as tc:
        with (
            tc.tile_pool(name="consts", bufs=1) as cpool,
            tc.tile_pool(name="big", bufs=1) as big,
            tc.tile_pool(name="xtp", bufs=XT_BUFS) as xtp,
            tc.tile_pool(name="expp", bufs=EXP_BUFS) as expp,
            tc.tile_pool(name="work", bufs=2) as work,
            tc.tile_pool(name="ps", bufs=1, space="PSUM") as ps,
        ):
            # ---- one-time prologue ----
            if timing:
                tick_sb = cpool.tile([1, 16], F32)
                nc.sync.dma_start(out=tick_sb, in_=tick[0:1, :])
                zs = cpool.tile([128, 2048], F32)
                nc.vector.memset(zs, 0.0)
                zs_bf = zs.bitcast(BF16)  # [128, 4096] bf16 zeros
                for pb in range(DC):
                    nc.sync.dma_start(
                        out=xT[pb * 128 : (pb + 1) * 128, :], in_=zs_bf
                    )
                for w in (wq, wk, wv):
                    nc.sync.dma_start(out=w[:, :], in_=zs_bf[:, : DC * H])

            wq_sb = cpool.tile([128, DC, H], BF16)
            nc.sync.dma_start(out=wq_sb, in_=wq.rearrange("p (c h) -> p c h", c=DC))
            wk_sb = cpool.tile([128, DC, H], BF16)
            nc.sync.dma_start(out=wk_sb, in_=wk.rearrange("p (c h) -> p c h", c=DC))
            wv_sb = cpool.tile([128, DC, H], BF16)
            nc.sync.dma_start(out=wv_sb, in_=wv.rearrange("p (c h) -> p c h", c=DC))
            ones_f32 = cpool.tile([128, 128], F32)
            nc.vector.memset(ones_f32, 1.0)
            ones_sb = cpool.tile([128, 128], BF16)
            nc.vector.tensor_copy(out=ones_sb, in_=ones_f32)
            # tiny dummy exp so the ACT table set loads before phase B
            warm = cpool.tile([1, 16], F32)
            nc.scalar.activation(
                warm, ones_f32[0:1, 0:16], mybir.ActivationFunctionType.Exp
            )
            # warm must have a reader for the BIR verifier; park it in a
            # DRAM scratch nothing else touches
            nc.sync.dma_start(out=warm_d[0:1, :], in_=warm)

            def body():
                # ---- persistent activations ----
                qT_sb = big.tile([128, SQ], BF16, name="qT_sb")
                kT_sb = big.tile([128, S], BF16, name="kT_sb")
                v_sb = big.tile([128, KB, 128], BF16, name="v_sb")  # [k,h] blocks
                accs = [
                    big.tile([128, 1024], BF16, name=f"acc{h}") for h in range(2)
                ]
                last_ex = {}
                oT_ps = [
                    ps.tile([128, 1024], F32, tag=f"oT{h}", bufs=1, name=f"oT{h}")
                    for h in range(2)
                ]

                xt_tiles = {}

                def emit_xt(nb):
                    xts = []
                    for dc in range(DC):
                        xt_t = xtp.tile([128, 512], BF16, tag="xt", name=f"xt{nb}_{dc}")
                        nc.sync.dma_start(
                            out=xt_t,
                            in_=xT[
                                dc * 128 : (dc + 1) * 128, nb * 512 : (nb + 1) * 512
                            ],
                        )
                        xts.append(xt_t)
                    xt_tiles[nb] = xts

                def emit_proj_qk(nb):
                    if nb not in xt_tiles:
                        emit_xt(nb)
                    if nb + 1 < NB and nb + 1 not in xt_tiles:
                        emit_xt(nb + 1)
                    xts = xt_tiles[nb]
                    cols = slice(nb * 512, (nb + 1) * 512)

                    if nb < SQ // 512:
                        qps = ps.tile(
                            [128, 1024], F32, tag="sc", bufs=SC_BUFS, name=f"qps{nb}"
                        )
                        for dc in range(DC):
                            nc.tensor.matmul(
                                qps[:, :512],
                                wq_sb[:, dc],
                                xts[dc],
                                start=dc == 0,
                                stop=dc == DC - 1,
                            )
                        nc.vector.tensor_copy(out=qT_sb[:, cols], in_=qps[:, :512])

                    kps = ps.tile(
                        [128, 1024], F32, tag="sc", bufs=SC_BUFS, name=f"kps{nb}"
                    )
                    for dc in range(DC):
                        nc.tensor.matmul(
                            kps[:, :512],
                            wk_sb[:, dc],
                            xts[dc],
                            start=dc == 0,
                            stop=dc == DC - 1,
                        )
                    nc.vector.tensor_copy(out=kT_sb[:, cols], in_=kps[:, :512])

                def emit_proj_v(nb):
                    # v natural [k, h]: xt chunks as stationary, wv moving.
                    # Same cycle count as the vT projection but no transpose.
                    xts = xt_tiles.pop(nb)
                    vps = ps.tile(
                        [128, 1024], F32, tag="sc", bufs=SC_BUFS, name=f"vps{nb}"
                    )
                    vps4 = vps[:, :512].rearrange("p (t h) -> p t h", t=4)
                    for t in range(4):
                        for dc in range(DC):
                            nc.tensor.matmul(
                                vps4[:, t],
                                xts[dc][:, t * 128 : (t + 1) * 128],
                                wv_sb[:, dc],
                                start=dc == 0,
                                stop=dc == DC - 1,
                            )
                    nc.vector.tensor_copy(
                        out=v_sb[:, nb * 4 : (nb + 1) * 4], in_=vps4
                    )

                def emit_unit_front(kb, h):
                    sc_t = ps.tile(
                        [128, 1024], F32, tag="sc", bufs=SC_BUFS, name=f"sc{h}_{kb}"
                    )
                    for c in range(2):
                        nc.tensor.matmul(
                            sc_t[:, c * 512 : (c + 1) * 512],
                            kT_sb[:, kb * 128 : (kb + 1) * 128],
                            qT_sb[:, h * 1024 + c * 512 : h * 1024 + (c + 1) * 512],
                            start=True,
                            stop=True,
                        )
                    ex = expp.tile([128, 1024], BF16, tag="ex", name=f"ex{h}_{kb}")
                    nc.scalar.activation(ex, sc_t, mybir.ActivationFunctionType.Exp)
                    return ex

                def emit_unit_back(kb, h, ex):
                    for c in range(2):
                        nc.tensor.matmul(
                            oT_ps[h][:, c * 512 : (c + 1) * 512],
                            v_sb[:, kb],
                            ex[:, c * 512 : (c + 1) * 512],
                            start=kb == 0,
                            stop=kb == KB - 1,
                        )
                    if kb == 0:
                        nc.vector.tensor_copy(out=accs[h], in_=ex)
                    elif kb < KB - 1:
                        nc.vector.tensor_add(accs[h], accs[h], ex)
                    else:
                        last_ex[h] = ex

                def emit_finale(h):
                    # den[q] = ones.T @ (acc + ex_last), accumulated in PSUM and
                    # replicated across partitions by the all-ones stationary
                    den_ps = ps.tile(
                        [128, 1024], F32, tag="sc", bufs=SC_BUFS, name=f"denps{h}"
                    )
                    for c in range(2):
                        cc = slice(c * 512, (c + 1) * 512)
                        nc.tensor.matmul(
                            den_ps[:, cc], ones_sb, accs[h][:, cc],
                            start=True, stop=False,
                        )
                        nc.tensor.matmul(
                            den_ps[:, cc], ones_sb, last_ex[h][:, cc],
                            start=False, stop=True,
                        )
                    recip = work.tile([128, 1024], F32, tag="recip", name=f"recip{h}")
                    onrm = work.tile([128, 1024], F32, tag="onrm", name=f"onrm{h}")
                    for c in range(2):
                        cc = slice(c * 512, (c + 1) * 512)
                        nc.vector.reciprocal_approx_fast(
                            out=recip[:, cc], in_=den_ps[:, cc]
                        )
                        nc.vector.tensor_mul(onrm[:, cc], oT_ps[h][:, cc], recip[:, cc])
                        nc.sync.dma_start(
                            out=outT[:, h * 1024 + c * 512 : h * 1024 + (c + 1) * 512],
                            in_=onrm[:, cc],
                        )

                # ---- interleaved emission plan ----
                # Rate-matched so ACT (exp; ~1.15us/unit) starts early and
                # never starves: QK of the first two blocks, then units as
                # kT/qT become available, V projections deferred into gaps.
                plan = [("qk", 0), ("qk", 1)]
                plan += [("unit", kb, 0) for kb in range(0, 4)]
                plan += [("v", 0), ("v", 1)]
                plan += [("unit", kb, 0) for kb in range(4, 8)]
                plan += [("qk", 2), ("v", 2)]
                plan += [("unit", kb, 0) for kb in range(8, 11)]
                plan += [("qk", 3), ("v", 3)]
                plan += [("unit", kb, 0) for kb in range(11, 14)]
                rest = []
                l0 = [(kb, 0) for kb in range(14, KB)]
                l1 = [(kb, 1) for kb in range(KB)]
                for i in range(max(len(l0), len(l1))):
                    if i < len(l1):
                        rest.append(l1[i])
                    if i < len(l0):
                        rest.append(l0[i])
                ri = 0
                for nb in range(4, NB):
                    plan += [("qk", nb), ("v", nb)]
                    take = 5
                    while take > 0 and ri < len(rest):
                        kb, h = rest[ri]
                        if kb >= 4 * nb:
                            break
                        plan.append(("unit", kb, h))
                        ri += 1
                        take -= 1
                while ri < len(rest):
                    plan.append(("unit", *rest[ri]))
                    ri += 1

                # software pipeline: back-half of unit i emitted after
                # front-half of unit i+2
                pend = []
                v_ready = 0  # kb blocks with v_sb written (4 per v-proj)

                def drain_ready(minpend):
                    while len(pend) > minpend and pend[0][0] < v_ready:
                        pkb, ph, pex = pend.pop(0)
                        emit_unit_back(pkb, ph, pex)
                        if pkb == KB - 1:
                            emit_finale(ph)

                for step in plan:
                    if step[0] == "qk":
                        emit_proj_qk(step[1])
                        continue
                    if step[0] == "v":
                        emit_proj_v(step[1])
                        v_ready = (step[1] + 1) * 4
                        drain_ready(2)
                        continue
                    _, kb, h = step
                    pend.append((kb, h, emit_unit_front(kb, h)))
                    drain_ready(2)
                drain_ready(0)

            if loop_n is not None:
                with tc.For_i(0, loop_n):
                    body()
            else:
                body()

    nc.compile()
    return nc


def build_attention_v3(loop_n=None, timing=False):
    """v2 + pair-wise K/V split: core 2b+j projects K/V only for its own
    2048 columns; the other half arrives via a 2-core AllGather (kT first,
    then v), overlapped with attention on the own half. Halves the
    projection PE work (~17us/core)."""
    from concourse.tile import TileContext

    nc = bacc.Bacc("TRN2", target_bir_lowering=False, debug=False, num_devices=8)

    kind_in = "Internal" if timing else "ExternalInput"
    xT = nc.dram_tensor("xT", (D, S), BF16, kind=kind_in)
    wq = nc.dram_tensor("wq", (128, DC * H), BF16, kind=kind_in)
    wk = nc.dram_tensor("wk", (128, DC * H), BF16, kind=kind_in)
    wv = nc.dram_tensor("wv", (128, DC * H), BF16, kind=kind_in)
    outT = nc.dram_tensor("outT", (H, SQ), F32, kind="ExternalOutput")
    tick = (
        nc.dram_tensor("tick", (1, 16), F32, kind="ExternalInput") if timing else None
    )
    warm_d = nc.dram_tensor("warm_d", (1, 16), F32, kind="Internal")
    kvo_k = nc.dram_tensor("kvo_k", (128, SQ), BF16, kind="Internal")
    kvg_k = nc.dram_tensor("kvg_k", (256, SQ), BF16, kind="Internal")
    kvo_v = nc.dram_tensor("kvo_v", (128, SQ), BF16, kind="Internal")
    kvg_v = nc.dram_tensor("kvg_v", (256, SQ), BF16, kind="Internal")
    GROUPS = [[0, 1], [2, 3], [4, 5], [6, 7]]

    with TileContext(nc) as tc:
        with (
            tc.tile_pool(name="consts", bufs=1) as cpool,
            tc.tile_pool(name="big", bufs=1) as big,
            tc.tile_pool(name="xtp", bufs=XT_BUFS) as xtp,
            tc.tile_pool(name="expp", bufs=EXP_BUFS) as expp,
            tc.tile_pool(name="work", bufs=2) as work,
            tc.tile_pool(name="ps", bufs=1, space="PSUM") as ps,
        ):
            if timing:
                tick_sb = cpool.tile([1, 16], F32)
                nc.sync.dma_start(out=tick_sb, in_=tick[0:1, :])
                zs = cpool.tile([128, 2048], F32)
                nc.vector.memset(zs, 0.0)
                zs_bf = zs.bitcast(BF16)
                for pb in range(DC):
                    nc.sync.dma_start(
                        out=xT[pb * 128 : (pb + 1) * 128, :], in_=zs_bf
                    )
                for w in (wq, wk, wv):
                    nc.sync.dma_start(out=w[:, :], in_=zs_bf[:, : DC * H])

            wq_sb = cpool.tile([128, DC, H], BF16)
            nc.sync.dma_start(out=wq_sb, in_=wq.rearrange("p (c h) -> p c h", c=DC))
            wk_sb = cpool.tile([128, DC, H], BF16)
            nc.sync.dma_start(out=wk_sb, in_=wk.rearrange("p (c h) -> p c h", c=DC))
            wv_sb = cpool.tile([128, DC, H], BF16)
            nc.sync.dma_start(out=wv_sb, in_=wv.rearrange("p (c h) -> p c h", c=DC))
            ones_f32 = cpool.tile([128, 128], F32)
            nc.vector.memset(ones_f32, 1.0)
            ones_sb = cpool.tile([128, 128], BF16)
            nc.vector.tensor_copy(out=ones_sb, in_=ones_f32)
            warm = cpool.tile([1, 16], F32)
            nc.scalar.activation(
                warm, ones_f32[0:1, 0:16], mybir.ActivationFunctionType.Exp
            )
            # warm must have a reader for the BIR verifier; park it in a
            # DRAM scratch nothing else touches
            nc.sync.dma_start(out=warm_d[0:1, :], in_=warm)

            def body():
                qT_sb = big.tile([128, SQ], BF16, name="qT_sb")
                kT_sb = big.tile([128, S], BF16, name="kT_sb")
                v_sb = big.tile([128, KB, 128], BF16, name="v_sb")
                accs = [
                    big.tile([128, 1024], BF16, name=f"acc{h}") for h in range(2)
                ]
                last_ex = {}
                oT_ps = [
                    ps.tile([128, 1024], F32, tag=f"oT{h}", bufs=1, name=f"oT{h}")
                    for h in range(2)
                ]

                xt_tiles = {}

                def emit_xt(nb):
                    xts = []
                    for dc in range(DC):
                        xt_t = xtp.tile([128, 512], BF16, tag="xt", name=f"xt{nb}_{dc}")
                        nc.sync.dma_start(
                            out=xt_t,
                            in_=xT[
                                dc * 128 : (dc + 1) * 128, nb * 512 : (nb + 1) * 512
                            ],
                        )
                        xts.append(xt_t)
                    xt_tiles[nb] = xts

                def emit_proj_qk(nb):
                    if nb not in xt_tiles:
                        emit_xt(nb)
                    if nb + 1 < 4 and nb + 1 not in xt_tiles:
                        emit_xt(nb + 1)
                    xts = xt_tiles[nb]
                    cols = slice(nb * 512, (nb + 1) * 512)
                    qps = ps.tile(
                        [128, 1024], F32, tag="sc", bufs=SC_BUFS, name=f"qps{nb}"
                    )
                    for dc in range(DC):
                        nc.tensor.matmul(
                            qps[:, :512], wq_sb[:, dc], xts[dc],
                            start=dc == 0, stop=dc == DC - 1,
                        )
                    nc.vector.tensor_copy(out=qT_sb[:, cols], in_=qps[:, :512])
                    kps = ps.tile(
                        [128, 1024], F32, tag="sc", bufs=SC_BUFS, name=f"kps{nb}"
                    )
                    for dc in range(DC):
                        nc.tensor.matmul(
                            kps[:, :512], wk_sb[:, dc], xts[dc],
                            start=dc == 0, stop=dc == DC - 1,
                        )
                    nc.vector.tensor_copy(out=kT_sb[:, cols], in_=kps[:, :512])

                def emit_proj_v(nb):
                    xts = xt_tiles.pop(nb)
                    vps = ps.tile(
                        [128, 1024], F32, tag="sc", bufs=SC_BUFS, name=f"vps{nb}"
                    )
                    vps4 = vps[:, :512].rearrange("p (t h) -> p t h", t=4)
                    for t in range(4):
                        for dc in range(DC):
                            nc.tensor.matmul(
                                vps4[:, t],
                                xts[dc][:, t * 128 : (t + 1) * 128],
                                wv_sb[:, dc],
                                start=dc == 0,
                                stop=dc == DC - 1,
                            )
                    nc.vector.tensor_copy(
                        out=v_sb[:, nb * 4 : (nb + 1) * 4], in_=vps4
                    )

                pid = nc.sync.partition_id()
                odd = pid % 2

                def emit_exchange_k():
                    nc.sync.dma_start(out=kvo_k[:, :], in_=kT_sb[:, :SQ])
                    nc.gpsimd.collective_compute(
                        "AllGather",
                        mybir.AluOpType.bypass,
                        replica_groups=GROUPS,
                        ins=[kvo_k[:, :]],
                        outs=[kvg_k[:, :]],
                    )
                    nc.sync.dma_start(
                        out=kT_sb[:, SQ:], in_=kvg_k[128:256, :], cond=(odd == 0)
                    )
                    nc.sync.dma_start(
                        out=kT_sb[:, SQ:], in_=kvg_k[0:128, :], cond=(odd == 1)
                    )

                def emit_exchange_v():
                    vflat = v_sb.rearrange("p b h -> p (b h)")
                    nc.sync.dma_start(out=kvo_v[:, :], in_=vflat[:, :SQ])
                    nc.gpsimd.collective_compute(
                        "AllGather",
                        mybir.AluOpType.bypass,
                        replica_groups=GROUPS,
                        ins=[kvo_v[:, :]],
                        outs=[kvg_v[:, :]],
                    )
                    nc.sync.dma_start(
                        out=vflat[:, SQ:], in_=kvg_v[128:256, :], cond=(odd == 0)
                    )
                    nc.sync.dma_start(
                        out=vflat[:, SQ:], in_=kvg_v[0:128, :], cond=(odd == 1)
                    )

                def emit_unit_front(kb, h):
                    sc_t = ps.tile(
                        [128, 1024], F32, tag="sc", bufs=SC_BUFS, name=f"sc{h}_{kb}"
                    )
                    for c in range(2):
                        nc.tensor.matmul(
                            sc_t[:, c * 512 : (c + 1) * 512],
                            kT_sb[:, kb * 128 : (kb + 1) * 128],
                            qT_sb[:, h * 1024 + c * 512 : h * 1024 + (c + 1) * 512],
                            start=True,
                            stop=True,
                        )
                    ex = expp.tile([128, 1024], BF16, tag="ex", name=f"ex{h}_{kb}")
                    nc.scalar.activation(ex, sc_t, mybir.ActivationFunctionType.Exp)
                    return ex

                def emit_unit_back(kb, h, ex):
                    for c in range(2):
                        nc.tensor.matmul(
                            oT_ps[h][:, c * 512 : (c + 1) * 512],
                            v_sb[:, kb],
                            ex[:, c * 512 : (c + 1) * 512],
                            start=kb == 0,
                            stop=kb == KB - 1,
                        )
                    if kb == 0:
                        nc.vector.tensor_copy(out=accs[h], in_=ex)
                    elif kb < KB - 1:
                        nc.vector.tensor_add(accs[h], accs[h], ex)
                    else:
                        last_ex[h] = ex

                def emit_finale(h):
                    den_ps = ps.tile(
                        [128, 1024], F32, tag="sc", bufs=SC_BUFS, name=f"denps{h}"
                    )
                    for c in range(2):
                        cc = slice(c * 512, (c + 1) * 512)
                        nc.tensor.matmul(
                            den_ps[:, cc], ones_sb, accs[h][:, cc],
                            start=True, stop=False,
                        )
                        nc.tensor.matmul(
                            den_ps[:, cc], ones_sb, last_ex[h][:, cc],
                            start=False, stop=True,
                        )
                    recip = work.tile([128, 1024], F32, tag="recip", name=f"recip{h}")
                    onrm = work.tile([128, 1024], F32, tag="onrm", name=f"onrm{h}")
                    for c in range(2):
                        cc = slice(c * 512, (c + 1) * 512)
                        nc.vector.reciprocal_approx_fast(
                            out=recip[:, cc], in_=den_ps[:, cc]
                        )
                        nc.vector.tensor_mul(onrm[:, cc], oT_ps[h][:, cc], recip[:, cc])
                        nc.sync.dma_start(
                            out=outT[:, h * 1024 + c * 512 : h * 1024 + (c + 1) * 512],
                            in_=onrm[:, cc],
                        )

                # ---- emission plan ----
                plan = [("qk", 0), ("qk", 1)]
                plan += [("unit", kb, 0) for kb in range(0, 3)]
                plan += [("v", 0)]
                plan += [("unit", kb, 0) for kb in range(3, 6)]
                plan += [("qk", 2)]
                plan += [("unit", kb, 0) for kb in range(6, 9)]
                plan += [("v", 1)]
                plan += [("unit", kb, 0) for kb in range(9, 11)]
                plan += [("qk", 3), ("xk",)]
                plan += [("unit", 11, 0), ("unit", 0, 1), ("unit", 12, 0)]
                plan += [("v", 2)]
                plan += [("unit", 1, 1), ("unit", 13, 0), ("unit", 2, 1)]
                plan += [("v", 3), ("xv",)]
                # remaining own units
                own = []
                for i in range(3, 16):
                    own.append((i, 1))
                    if i + 11 < 16:
                        own.append((i + 11, 0))
                # partner units, h=0 leading so finale(0) overlaps the tail
                p0 = [(kb, 0) for kb in range(16, KB)]
                p1 = [(kb, 1) for kb in range(16, KB)]
                partner = p0[:6]
                i0, i1 = 6, 0
                while i0 < len(p0) or i1 < len(p1):
                    if i0 < len(p0):
                        partner.append(p0[i0]); i0 += 1
                    if i1 < len(p1):
                        partner.append(p1[i1]); i1 += 1
                plan += [("unit", kb, h) for kb, h in own + partner]

                pend = []
                v_ready = 0

                def drain_ready(minpend):
                    while len(pend) > minpend and pend[0][0] < v_ready:
                        pkb, ph, pex = pend.pop(0)
                        emit_unit_back(pkb, ph, pex)
                        if pkb == KB - 1:
                            emit_finale(ph)

                for step in plan:
                    if step[0] == "qk":
                        emit_proj_qk(step[1])
                    elif step[0] == "v":
                        emit_proj_v(step[1])
                        v_ready = (step[1] + 1) * 4
                        drain_ready(2)
                    elif step[0] == "xk":
                        emit_exchange_k()
                    elif step[0] == "xv":
                        emit_exchange_v()
                        v_ready = KB
                        drain_ready(2)
                    else:
                        _, kb, h = step
                        pend.append((kb, h, emit_unit_front(kb, h)))
                        drain_ready(2)
                drain_ready(0)

            if loop_n is not None:
                with tc.For_i(0, loop_n):
                    body()
            else:
                body()

    nc.compile()
    return nc


_NC_CACHE = None


def _get_nc():
    global _NC_CACHE
    if _NC_CACHE is None:
        _NC_CACHE = build_attention_v2()
    return _NC_CACHE


def kernel(x, Wq, Wk, Wv):
    x = np.asarray(x, dtype=np.float32)
    Wq = np.asarray(Wq, dtype=np.float32)
    Wk = np.asarray(Wk, dtype=np.float32)
    Wv = np.asarray(Wv, dtype=np.float32)
    assert x.shape == (B, S, D), x.shape

    bf = ml_dtypes.bfloat16

    def warr(w):
        # [D, H] -> [128p, DC*H] with row p = [chunk0 h..., chunk1 h...]
        return np.ascontiguousarray(
            w.reshape(DC, 128, H).transpose(1, 0, 2).reshape(128, DC * H).astype(bf)
        )

    wq = warr(Wq / np.sqrt(np.float32(H)))
    wk = warr(Wk)
    wv = warr(Wv)
    in_maps = []
    for c in range(8):
        bi, hq = divmod(c, 2)
        xt = x[bi].T  # [d, s]
        if hq == 1:
            xt = np.concatenate([xt[:, SQ:], xt[:, :SQ]], axis=1)
        in_maps.append(
            {
                "xT": np.ascontiguousarray(xt.astype(bf)),
                "wq": wq,
                "wk": wk,
                "wv": wv,
            }
        )

    nc = _get_nc()
    res = run_bass_kernel_spmd(nc, in_maps, core_ids=list(range(8)))

    out = np.empty((B, S, H), dtype=np.float32)
    for c in range(8):
        bi, hq = divmod(c, 2)
        out[bi, hq * SQ : (hq + 1) * SQ] = res.results[c]["outT"].T
    return out


if __name__ == "__main__":
    rng = np.random.default_rng(0)
    x = rng.standard_normal((B, S, D), dtype=np.float32)
    s = 1.0 / np.sqrt(D)
    Wq = rng.standard_normal((D, H), dtype=np.float32) * s
    Wk = rng.standard_normal((D, H), dtype=np.float32) * s
    Wv = rng.standard_normal((D, H), dtype=np.float32) * s
    out = kernel(x=x, Wq=Wq, Wk=Wk, Wv=Wv)
    print("out", out.shape, out.dtype, float(np.abs(out).max()))


# revision 22
# speedup vs baseline: 1.2176x; 1.2176x over previous
"""Single-head attention (b=4, s=4096, d=1024, h=128) on 8 Trainium2 NeuronCores.

Sharding: data-parallel over batch x query-halves -> 8 independent cores
(core c handles batch c//2, query rows [hq*2048, (hq+1)*2048) with hq = c%2).
K/V work is replicated per batch pair; no collectives.

Host prep per core: x[b].T (d-major) in bf16, sequence columns rotated so
the core's 2048 query rows come first (softmax over keys is permutation-
invariant); weights in bf16, pre-arranged to [128p, 8c*128h] rows so their
DMA runs 2KB-contiguous descriptors; 1/sqrt(h) folded into Wq. The kernel
returns outT [h, 2048] f32 per core; the host transposes back.

Device kernel (bf16 matmul operands, fp32 PSUM accumulation):
  projections (per 512-col block, interleaved with attention units):
    qT = wq.T @ xT [128h, 2048q]; kT = wk.T @ xT [128h, 4096k]
    v natural [k,h] computed directly (xt chunks stationary, wv moving) --
    no transpose pass; PSUM->SBUF copies on DVE (bf16 out), ACT exp-only
  attention unit (key-block kb, q-half h); both halves' oT live in PSUM:
    sc = kT[kb].T @ qT[half] [128k, 1024q] PSUM f32 (2x 512-wide matmuls)
    ex = exp(sc)  -- ONE 1024-wide ACT instr (table preloaded at t=0)
    oT[half] += v[kb].T' @ ex  (PSUM accumulate over kb)
    den: DVE bf16 chain-accumulate acc[h] += ex (last block folded into
    the den matmul instead)
  finale per half: den = ones.T @ (acc + ex_last) on PE (partition sum,
    broadcast via the all-ones stationary); DVE reciprocal+multiply and
    output DMA in 512-col chunks.

The emission plan rate-matches PE and ACT: exp (the co-bottleneck, ~73us
of ACT) starts ~14us in, V projections are deferred into pipeline gaps
(back-units gated on v-block availability), and the two q-halves finish
staggered so the first finale overlaps the second half's tail. Measured
steady-state: ~112us/core vs 179us for the fp32r two-phase baseline under
the same loop harness (rel err 1.01e-2 vs reference, gate 2e-2).
"""

import sys

sys.path.insert(0, "/opt/trn_rl_repo")

import numpy as np
import ml_dtypes

import concourse.mybir as mybir
from concourse import bacc
from concourse.bass_utils import run_bass_kernel_spmd

F32 = mybir.dt.float32
BF16 = mybir.dt.bfloat16

B = 4  # batch
D = 1024  # d_model
H = 128  # head size
S = 4096  # full sequence (keys)
SQ = 2048  # queries per core
DC = D // 128  # 8 d-chunks
NB = S // 512  # 8 column blocks for projections
KB = S // 128  # 32 key blocks
SC_BUFS = 2
EXP_BUFS = 8
XT_BUFS = 24


def build_attention_v2(loop_n=None, timing=False):
    """Build the v2 attention kernel; see module docstring.

    loop_n/timing wrap the body in an on-device For_i loop with Internal
    (device-resident, zero-filled) inputs for steady-state benchmarking.
    """
    from concourse.tile import TileContext

    nc = bacc.Bacc("TRN2", target_bir_lowering=False, debug=False)

    kind_in = "Internal" if timing else "ExternalInput"
    xT = nc.dram_tensor("xT", (D, S), BF16, kind=kind_in)
    # weights pre-arranged on host to [128p, DC*H] so the DMA runs 2KB rows
    wq = nc.dram_tensor("wq", (128, DC * H), BF16, kind=kind_in)
    wk = nc.dram_tensor("wk", (128, DC * H), BF16, kind=kind_in)
    wv = nc.dram_tensor("wv", (128, DC * H), BF16, kind=kind_in)
    outT = nc.dram_tensor("outT", (H, SQ), F32, kind="ExternalOutput")
    tick = (
        nc.dram_tensor("tick", (1, 16), F32, kind="ExternalInput") if timing else None
    )
    warm_d = nc.dram_tensor("warm_d", (1, 16), F32, kind="Internal")

    with TileContext(nc) as tc:
        with (
            tc.tile_pool(name="consts", bufs=1) as cpool,
            tc.tile_pool(name="big", bufs=1) as big,
            tc.tile_pool(name="xtp", bufs=XT_BUFS) as xtp,
            tc.tile_pool(name="expp", bufs=EXP_BUFS) as expp,
            tc.tile_pool(name="work", bufs=2) as work,
            tc.tile_pool(name="ps", bufs=1, space="PSUM") as ps,
        ):
            # ---- one-time prologue ----
            if timing:
                tick_sb = cpool.tile([1, 16], F32)
                nc.sync.dma_start(out=tick_sb, in_=tick[0:1, :])
                zs = cpool.tile([128, 2048], F32)
                nc.vector.memset(zs, 0.0)
                zs_bf = zs.bitcast(BF16)  # [128, 4096] bf16 zeros
                for pb in range(DC):
                    nc.sync.dma_start(
                        out=xT[pb * 128 : (pb + 1) * 128, :], in_=zs_bf
                    )
                for w in (wq, wk, wv):
                    nc.sync.dma_start(out=w[:, :], in_=zs_bf[:, : DC * H])

            wq_sb = cpool.tile([128, DC, H], BF16)
            nc.sync.dma_start(out=wq_sb, in_=wq.rearrange("p (c h) -> p c h", c=DC))
            wk_sb = cpool.tile([128, DC, H], BF16)
            nc.sync.dma_start(out=wk_sb, in_=wk.rearrange("p (c h) -> p c h", c=DC))
            wv_sb = cpool.tile([128, DC, H], BF16)
            nc.sync.dma_start(out=wv_sb, in_=wv.rearrange("p (c h) -> p c h", c=DC))
            ones_f32 = cpool.tile([128, 128], F32)
            nc.vector.memset(ones_f32, 1.0)
            ones_sb = cpool.tile([128, 128], BF16)
            nc.vector.tensor_copy(out=ones_sb, in_=ones_f32)
            # tiny dummy exp so the ACT table set loads before phase B
            warm = cpool.tile([1, 16], F32)
            nc.scalar.activation(
                warm, ones_f32[0:1, 0:16], mybir.ActivationFunctionType.Exp
            )
            # warm must have a reader for the BIR verifier; park it in a
            # DRAM scratch nothing else touches
            nc.sync.dma_start(out=warm_d[0:1, :], in_=warm)

            def body():
                # ---- persistent activations ----
                qT_sb = big.tile([128, SQ], BF16, name="qT_sb")
                kT_sb = big.tile([128, S], BF16, name="kT_sb")
                v_sb = big.tile([128, KB, 128], BF16, name="v_sb")  # [k,h] blocks
                accs = [
                    big.tile([128, 1024], BF16, name=f"acc{h}") for h in range(2)
                ]
                last_ex = {}
                oT_ps = [
                    ps.tile([128, 1024], F32, tag=f"oT{h}", bufs=1, name=f"oT{h}")
                    for h in range(2)
                ]

                xt_tiles = {}

                def emit_xt(nb):
                    xts = []
                    for dc in range(DC):
                        xt_t = xtp.tile([128, 512], BF16, tag="xt", name=f"xt{nb}_{dc}")
                        nc.sync.dma_start(
                            out=xt_t,
                            in_=xT[
                                dc * 128 : (dc + 1) * 128, nb * 512 : (nb + 1) * 512
                            ],
                        )
                        xts.append(xt_t)
                    xt_tiles[nb] = xts

                def emit_proj_qk(nb):
                    if nb not in xt_tiles:
                        emit_xt(nb)
                    if nb + 1 < NB and nb + 1 not in xt_tiles:
                        emit_xt(nb + 1)
                    xts = xt_tiles[nb]
                    cols = slice(nb * 512, (nb + 1) * 512)

                    if nb < SQ // 512:
                        qps = ps.tile(
                            [128, 1024], F32, tag="sc", bufs=SC_BUFS, name=f"qps{nb}"
                        )
                        for dc in range(DC):
                            nc.tensor.matmul(
                                qps[:, :512],
                                wq_sb[:, dc],
                                xts[dc],
                                start=dc == 0,
                                stop=dc == DC - 1,
                            )
                        nc.vector.tensor_copy(out=qT_sb[:, cols], in_=qps[:, :512])

                    kps = ps.tile(
                        [128, 1024], F32, tag="sc", bufs=SC_BUFS, name=f"kps{nb}"
                    )
                    for dc in range(DC):
                        nc.tensor.matmul(
                            kps[:, :512],
                            wk_sb[:, dc],
                            xts[dc],
                            start=dc == 0,
                            stop=dc == DC - 1,
                        )
                    nc.vector.tensor_copy(out=kT_sb[:, cols], in_=kps[:, :512])

                def emit_proj_v(nb):
                    # v natural [k, h]: xt chunks as stationary, wv moving.
                    # Same cycle count as the vT projection but no transpose.
                    xts = xt_tiles.pop(nb)
                    vps = ps.tile(
                        [128, 1024], F32, tag="sc", bufs=SC_BUFS, name=f"vps{nb}"
                    )
                    vps4 = vps[:, :512].rearrange("p (t h) -> p t h", t=4)
                    for t in range(4):
                        for dc in range(DC):
                            nc.tensor.matmul(
                                vps4[:, t],
                                xts[dc][:, t * 128 : (t + 1) * 128],
                                wv_sb[:, dc],
                                start=dc == 0,
                                stop=dc == DC - 1,
                            )
                    nc.vector.tensor_copy(
                        out=v_sb[:, nb * 4 : (nb + 1) * 4], in_=vps4
                    )

                def emit_unit_front(kb, h):
                    sc_t = ps.tile(
                        [128, 1024], F32, tag="sc", bufs=SC_BUFS, name=f"sc{h}_{kb}"
                    )
                    for c in range(2):
                        nc.tensor.matmul(
                            sc_t[:, c * 512 : (c + 1) * 512],
                            kT_sb[:, kb * 128 : (kb + 1) * 128],
                            qT_sb[:, h * 1024 + c * 512 : h * 1024 + (c + 1) * 512],
                            start=True,
                            stop=True,
                        )
                    ex = expp.tile([128, 1024], BF16, tag="ex", name=f"ex{h}_{kb}")
                    nc.scalar.activation(ex, sc_t, mybir.ActivationFunctionType.Exp)
                    return ex

                def emit_unit_back(kb, h, ex):
                    for c in range(2):
                        nc.tensor.matmul(
                            oT_ps[h][:, c * 512 : (c + 1) * 512],
                            v_sb[:, kb],
                            ex[:, c * 512 : (c + 1) * 512],
                            start=kb == 0,
                            stop=kb == KB - 1,
                        )
                    if kb == 0:
                        nc.vector.tensor_copy(out=accs[h], in_=ex)
                    elif kb < KB - 1:
                        nc.vector.tensor_add(accs[h], accs[h], ex)
                    else:
                        last_ex[h] = ex

                def emit_finale(h):
                    # den[q] = ones.T @ (acc + ex_last), accumulated in PSUM and
                    # replicated across partitions by the all-ones stationary
                    den_ps = ps.tile(
                        [128, 1024], F32, tag="sc", bufs=SC_BUFS, name=f"denps{h}"
                    )
                    recip = work.tile([128, 1024], F32, tag="recip", name=f"recip{h}")
                    onrm = work.tile([128, 1024], F32, tag="onrm", name=f"onrm{h}")
                    for c in range(2):
                        cc = slice(c * 512, (c + 1) * 512)
                        nc.tensor.matmul(
                            den_ps[:, cc], ones_sb, accs[h][:, cc],
                            start=True, stop=False,
                        )
                        nc.tensor.matmul(
                            den_ps[:, cc], ones_sb, last_ex[h][:, cc],
                            start=False, stop=True,
                        )
                        nc.vector.reciprocal_approx_fast(
                            out=recip[:, cc], in_=den_ps[:, cc]
                        )
                        nc.vector.tensor_mul(onrm[:, cc], oT_ps[h][:, cc], recip[:, cc])
                        nc.sync.dma_start(
                            out=outT[:, h * 1024 + c * 512 : h * 1024 + (c + 1) * 512],
                            in_=onrm[:, cc],
                        )

                # ---- interleaved emission plan ----
                # Rate-matched so ACT (exp; ~1.15us/unit) starts early and
                # never starves: QK of the first two blocks, then units as
                # kT/qT become available, V projections deferred into gaps.
                plan = [("qk", 0), ("qk", 1)]
                plan += [("unit", kb, 0) for kb in range(0, 4)]
                plan += [("v", 0), ("v", 1)]
                plan += [("unit", kb, 0) for kb in range(4, 8)]
                plan += [("qk", 2), ("v", 2)]
                plan += [("unit", kb, 0) for kb in range(8, 11)]
                plan += [("qk", 3), ("v", 3)]
                plan += [("unit", kb, 0) for kb in range(11, 14)]
                rest = []
                l0 = [(kb, 0) for kb in range(14, KB)]
                l1 = [(kb, 1) for kb in range(KB)]
                for i in range(max(len(l0), len(l1))):
                    if i < len(l1):
                        rest.append(l1[i])
                    if i < len(l0):
                        rest.append(l0[i])
                ri = 0
                for nb in range(4, NB):
                    plan += [("qk", nb), ("v", nb)]
                    take = 6
                    while take > 0 and ri < len(rest):
                        kb, h = rest[ri]
                        if kb >= 4 * nb:
                            break
                        plan.append(("unit", kb, h))
                        ri += 1
                        take -= 1
                while ri < len(rest):
                    plan.append(("unit", *rest[ri]))
                    ri += 1

                # software pipeline: back-half of unit i emitted after
                # front-half of unit i+2
                pend = []
                v_ready = 0  # kb blocks with v_sb written (4 per v-proj)

                def drain_ready(minpend):
                    while len(pend) > minpend and pend[0][0] < v_ready:
                        pkb, ph, pex = pend.pop(0)
                        emit_unit_back(pkb, ph, pex)
                        if pkb == KB - 1:
                            emit_finale(ph)

                for step in plan:
                    if step[0] == "qk":
                        emit_proj_qk(step[1])
                        continue
                    if step[0] == "v":
                        emit_proj_v(step[1])
                        v_ready = (step[1] + 1) * 4
                        drain_ready(2)
                        continue
                    _, kb, h = step
                    pend.append((kb, h, emit_unit_front(kb, h)))
                    drain_ready(2)
                drain_ready(0)

            if loop_n is not None:
                with tc.For_i(0, loop_n):
                    body()
            else:
                body()

    nc.compile()
    return nc


def build_attention_v3(loop_n=None, timing=False):
    """v2 + pair-wise K/V split: core 2b+j projects K/V only for its own
    2048 columns; the other half arrives via a 2-core AllGather (kT first,
    then v), overlapped with attention on the own half. Halves the
    projection PE work (~17us/core)."""
    from concourse.tile import TileContext

    nc = bacc.Bacc("TRN2", target_bir_lowering=False, debug=False, num_devices=8)

    kind_in = "Internal" if timing else "ExternalInput"
    xT = nc.dram_tensor("xT", (D, S), BF16, kind=kind_in)
    wq = nc.dram_tensor("wq", (128, DC * H), BF16, kind=kind_in)
    wk = nc.dram_tensor("wk", (128, DC * H), BF16, kind=kind_in)
    wv = nc.dram_tensor("wv", (128, DC * H), BF16, kind=kind_in)
    outT = nc.dram_tensor("outT", (H, SQ), F32, kind="ExternalOutput")
    tick = (
        nc.dram_tensor("tick", (1, 16), F32, kind="ExternalInput") if timing else None
    )
    warm_d = nc.dram_tensor("warm_d", (1, 16), F32, kind="Internal")
    kvo_k = nc.dram_tensor("kvo_k", (128, SQ), BF16, kind="Internal")
    kvg_k = nc.dram_tensor("kvg_k", (256, SQ), BF16, kind="Internal")
    kvo_v = nc.dram_tensor("kvo_v", (128, SQ), BF16, kind="Internal")
    kvg_v = nc.dram_tensor("kvg_v", (256, SQ), BF16, kind="Internal")
    GROUPS = [[0, 1], [2, 3], [4, 5], [6, 7]]

    with TileContext(nc) as tc:
        with (
            tc.tile_pool(name="consts", bufs=1) as cpool,
            tc.tile_pool(name="big", bufs=1) as big,
            tc.tile_pool(name="xtp", bufs=XT_BUFS) as xtp,
            tc.tile_pool(name="expp", bufs=EXP_BUFS) as expp,
            tc.tile_pool(name="work", bufs=2) as work,
            tc.tile_pool(name="ps", bufs=1, space="PSUM") as ps,
        ):
            if timing:
                tick_sb = cpool.tile([1, 16], F32)
                nc.sync.dma_start(out=tick_sb, in_=tick[0:1, :])
                zs = cpool.tile([128, 2048], F32)
                nc.vector.memset(zs, 0.0)
                zs_bf = zs.bitcast(BF16)
                for pb in range(DC):
                    nc.sync.dma_start(
                        out=xT[pb * 128 : (pb + 1) * 128, :], in_=zs_bf
                    )
                for w in (wq, wk, wv):
                    nc.sync.dma_start(out=w[:, :], in_=zs_bf[:, : DC * H])

            wq_sb = cpool.tile([128, DC, H], BF16)
            nc.sync.dma_start(out=wq_sb, in_=wq.rearrange("p (c h) -> p c h", c=DC))
            wk_sb = cpool.tile([128, DC, H], BF16)
            nc.sync.dma_start(out=wk_sb, in_=wk.rearrange("p (c h) -> p c h", c=DC))
            wv_sb = cpool.tile([128, DC, H], BF16)
            nc.sync.dma_start(out=wv_sb, in_=wv.rearrange("p (c h) -> p c h", c=DC))
            ones_f32 = cpool.tile([128, 128], F32)
            nc.vector.memset(ones_f32, 1.0)
            ones_sb = cpool.tile([128, 128], BF16)
            nc.vector.tensor_copy(out=ones_sb, in_=ones_f32)
            warm = cpool.tile([1, 16], F32)
            nc.scalar.activation(
                warm, ones_f32[0:1, 0:16], mybir.ActivationFunctionType.Exp
            )
            # warm must have a reader for the BIR verifier; park it in a
            # DRAM scratch nothing else touches
            nc.sync.dma_start(out=warm_d[0:1, :], in_=warm)

            def body():
                qT_sb = big.tile([128, SQ], BF16, name="qT_sb")
                kT_sb = big.tile([128, S], BF16, name="kT_sb")
                v_sb = big.tile([128, KB, 128], BF16, name="v_sb")
                accs = [
                    big.tile([128, 1024], BF16, name=f"acc{h}") for h in range(2)
                ]
                last_ex = {}
                oT_ps = [
                    ps.tile([128, 1024], F32, tag=f"oT{h}", bufs=1, name=f"oT{h}")
                    for h in range(2)
                ]

                xt_tiles = {}

                def emit_xt(nb):
                    xts = []
                    for dc in range(DC):
                        xt_t = xtp.tile([128, 512], BF16, tag="xt", name=f"xt{nb}_{dc}")
                        nc.sync.dma_start(
                            out=xt_t,
                            in_=xT[
                                dc * 128 : (dc + 1) * 128, nb * 512 : (nb + 1) * 512
                            ],
                        )
                        xts.append(xt_t)
                    xt_tiles[nb] = xts

                def emit_proj_qk(nb):
                    if nb not in xt_tiles:
                        emit_xt(nb)
                    if nb + 1 < 4 and nb + 1 not in xt_tiles:
                        emit_xt(nb + 1)
                    xts = xt_tiles[nb]
                    cols = slice(nb * 512, (nb + 1) * 512)
                    qps = ps.tile(
                        [128, 1024], F32, tag="sc", bufs=SC_BUFS, name=f"qps{nb}"
                    )
                    for dc in range(DC):
                        nc.tensor.matmul(
                            qps[:, :512], wq_sb[:, dc], xts[dc],
                            start=dc == 0, stop=dc == DC - 1,
                        )
                    nc.vector.tensor_copy(out=qT_sb[:, cols], in_=qps[:, :512])
                    kps = ps.tile(
                        [128, 1024], F32, tag="sc", bufs=SC_BUFS, name=f"kps{nb}"
                    )
                    for dc in range(DC):
                        nc.tensor.matmul(
                            kps[:, :512], wk_sb[:, dc], xts[dc],
                            start=dc == 0, stop=dc == DC - 1,
                        )
                    nc.vector.tensor_copy(out=kT_sb[:, cols], in_=kps[:, :512])

                def emit_proj_v(nb):
                    xts = xt_tiles.pop(nb)
                    vps = ps.tile(
                        [128, 1024], F32, tag="sc", bufs=SC_BUFS, name=f"vps{nb}"
                    )
                    vps4 = vps[:, :512].rearrange("p (t h) -> p t h", t=4)
                    for t in range(4):
                        for dc in range(DC):
                            nc.tensor.matmul(
                                vps4[:, t],
                                xts[dc][:, t * 128 : (t + 1) * 128],
                                wv_sb[:, dc],
                                start=dc == 0,
                                stop=dc == DC - 1,
                            )
                    nc.vector.tensor_copy(
                        out=v_sb[:, nb * 4 : (nb + 1) * 4], in_=vps4
                    )

                pid = nc.sync.partition_id()
                odd = pid % 2

                def emit_exchange_k():
                    nc.sync.dma_start(out=kvo_k[:, :], in_=kT_sb[:, :SQ])
                    nc.gpsimd.collective_compute(
                        "AllGather",
                        mybir.AluOpType.bypass,
                        replica_groups=GROUPS,
                        ins=[kvo_k[:, :]],
                        outs=[kvg_k[:, :]],
                    )
                    nc.sync.dma_start(
                        out=kT_sb[:, SQ:], in_=kvg_k[128:256, :], cond=(odd == 0)
                    )
                    nc.sync.dma_start(
                        out=kT_sb[:, SQ:], in_=kvg_k[0:128, :], cond=(odd == 1)
                    )

                def emit_exchange_v():
                    vflat = v_sb.rearrange("p b h -> p (b h)")
                    nc.sync.dma_start(out=kvo_v[:, :], in_=vflat[:, :SQ])
                    nc.gpsimd.collective_compute(
                        "AllGather",
                        mybir.AluOpType.bypass,
                        replica_groups=GROUPS,
                        ins=[kvo_v[:, :]],
                        outs=[kvg_v[:, :]],
                    )
                    nc.sync.dma_start(
                        out=vflat[:, SQ:], in_=kvg_v[128:256, :], cond=(odd == 0)
                    )
                    nc.sync.dma_start(
                        out=vflat[:, SQ:], in_=kvg_v[0:128, :], cond=(odd == 1)
                    )

                def emit_unit_front(kb, h):
                    sc_t = ps.tile(
                        [128, 1024], F32, tag="sc", bufs=SC_BUFS, name=f"sc{h}_{kb}"
                    )
                    for c in range(2):
                        nc.tensor.matmul(
                            sc_t[:, c * 512 : (c + 1) * 512],
                            kT_sb[:, kb * 128 : (kb + 1) * 128],
                            qT_sb[:, h * 1024 + c * 512 : h * 1024 + (c + 1) * 512],
                            start=True,
                            stop=True,
                        )
                    ex = expp.tile([128, 1024], BF16, tag="ex", name=f"ex{h}_{kb}")
                    nc.scalar.activation(ex, sc_t, mybir.ActivationFunctionType.Exp)
                    return ex

                def emit_unit_back(kb, h, ex):
                    for c in range(2):
                        nc.tensor.matmul(
                            oT_ps[h][:, c * 512 : (c + 1) * 512],
                            v_sb[:, kb],
                            ex[:, c * 512 : (c + 1) * 512],
                            start=kb == 0,
                            stop=kb == KB - 1,
                        )
                    if kb == 0:
                        nc.vector.tensor_copy(out=accs[h], in_=ex)
                    elif kb < KB - 1:
                        nc.vector.tensor_add(accs[h], accs[h], ex)
                    else:
                        last_ex[h] = ex

                def emit_finale(h):
                    den_ps = ps.tile(
                        [128, 1024], F32, tag="sc", bufs=SC_BUFS, name=f"denps{h}"
                    )
                    recip = work.tile([128, 1024], F32, tag="recip", name=f"recip{h}")
                    onrm = work.tile([128, 1024], F32, tag="onrm", name=f"onrm{h}")
                    for c in range(2):
                        cc = slice(c * 512, (c + 1) * 512)
                        nc.tensor.matmul(
                            den_ps[:, cc], ones_sb, accs[h][:, cc],
                            start=True, stop=False,
                        )
                        nc.tensor.matmul(
                            den_ps[:, cc], ones_sb, last_ex[h][:, cc],
                            start=False, stop=True,
                        )
                        nc.vector.reciprocal_approx_fast(
                            out=recip[:, cc], in_=den_ps[:, cc]
                        )
                        nc.vector.tensor_mul(onrm[:, cc], oT_ps[h][:, cc], recip[:, cc])
                        nc.sync.dma_start(
                            out=outT[:, h * 1024 + c * 512 : h * 1024 + (c + 1) * 512],
                            in_=onrm[:, cc],
                        )

                # ---- emission plan ----
                plan = [("qk", 0), ("qk", 1)]
                plan += [("unit", kb, 0) for kb in range(0, 3)]
                plan += [("v", 0)]
                plan += [("unit", kb, 0) for kb in range(3, 6)]
                plan += [("qk", 2)]
                plan += [("unit", kb, 0) for kb in range(6, 9)]
                plan += [("v", 1)]
                plan += [("unit", kb, 0) for kb in range(9, 11)]
                plan += [("qk", 3), ("xk",)]
                plan += [("unit", 11, 0), ("unit", 0, 1), ("unit", 12, 0)]
                plan += [("v", 2)]
                plan += [("unit", 1, 1), ("unit", 13, 0), ("unit", 2, 1)]
                plan += [("v", 3), ("xv",)]
                # remaining own units
                own = []
                for i in range(3, 16):
                    own.append((i, 1))
                    if i + 11 < 16:
                        own.append((i + 11, 0))
                # partner units, h=0 leading so finale(0) overlaps the tail
                p0 = [(kb, 0) for kb in range(16, KB)]
                p1 = [(kb, 1) for kb in range(16, KB)]
                partner = p0[:6]
                i0, i1 = 6, 0
                while i0 < len(p0) or i1 < len(p1):
                    if i0 < len(p0):
                        partner.append(p0[i0]); i0 += 1
                    if i1 < len(p1):
                        partner.append(p1[i1]); i1 += 1
                plan += [("unit", kb, h) for kb, h in own + partner]

                pend = []
                v_ready = 0

                def drain_ready(minpend):
                    while len(pend) > minpend and pend[0][0] < v_ready:
                        pkb, ph, pex = pend.pop(0)
                        emit_unit_back(pkb, ph, pex)
                        if pkb == KB - 1:
                            emit_finale(ph)

                for step in plan:
                    if step[0] == "qk":
                        emit_proj_qk(step[1])
                    elif step[0] == "v":
                        emit_proj_v(step[1])
                        v_ready = (step[1] + 1) * 4
                        drain_ready(2)
                    elif step[0] == "xk":
                        emit_exchange_k()
                    elif step[0] == "xv":
                        emit_exchange_v()
                        v_ready = KB
                        drain_ready(2)
                    else:
                        _, kb, h = step
                        pend.append((kb, h, emit_unit_front(kb, h)))
                        drain_ready(2)
                drain_ready(0)

            if loop_n is not None:
                with tc.For_i(0, loop_n):
                    body()
            else:
                body()

    nc.compile()
    return nc


_NC_CACHE = None


def _get_nc():
    global _NC_CACHE
    if _NC_CACHE is None:
        _NC_CACHE = build_attention_v2()
    return _NC_CACHE


def kernel(x, Wq, Wk, Wv):
    x = np.asarray(x, dtype=np.float32)
    Wq = np.asarray(Wq, dtype=np.float32)
    Wk = np.asarray(Wk, dtype=np.float32)
    Wv = np.asarray(Wv, dtype=np.float32)
    assert x.shape == (B, S, D), x.shape

    bf = ml_dtypes.bfloat16

    def warr(w):
        # [D, H] -> [128p, DC*H] with row p = [chunk0 h..., chunk1 h...]
        return np.ascontiguousarray(
            w.reshape(DC, 128, H).transpose(1, 0, 2).reshape(128, DC * H).astype(bf)
        )

    wq = warr(Wq / np.sqrt(np.float32(H)))
    wk = warr(Wk)
    wv = warr(Wv)
    in_maps = []
    for c in range(8):
        bi, hq = divmod(c, 2)
        xt = x[bi].T  # [d, s]
        if hq == 1:
            xt = np.concatenate([xt[:, SQ:], xt[:, :SQ]], axis=1)
        in_maps.append(
            {
                "xT": np.ascontiguousarray(xt.astype(bf)),
                "wq": wq,
                "wk": wk,
                "wv": wv,
            }
        )

    nc = _get_nc()
    res = run_bass_kernel_spmd(nc, in_maps, core_ids=list(range(8)))

    out = np.empty((B, S, H), dtype=np.float32)
    for c in range(8):
        bi, hq = divmod(c, 2)
        out[bi, hq * SQ : (hq + 1) * SQ] = res.results[c]["outT"].T
    return out


if __name__ == "__main__":
    rng = np.random.default_rng(0)
    x = rng.standard_normal((B, S, D), dtype=np.float32)
    s = 1.0 / np.sqrt(D)
    Wq = rng.standard_normal((D, H), dtype=np.float32) * s
    Wk = rng.standard_normal((D, H), dtype=np.float32) * s
    Wv = rng.standard_normal((D, H), dtype=np.float32) * s
    out = kernel(x=x, Wq=Wq, Wk=Wk, Wv=Wv)
    print("out", out.shape, out.dtype, float(np.abs(out).max()))
